# revision 15
# baseline (speedup 1.0000x reference)
"""Trainium2 Bass kernel v2 for nn_CrossAttnLayer (post-LN decoder layer:
self-attn + cross-attn + FFN).  B=4, S=M=1024, D=512, nhead=8, H=2048.

Sharding: 8 cores = (batch b = core//2) x (query-row half = core%2); each core
computes R=512 query rows of one batch end-to-end.  No collectives.

v2 design (vs baseline):
- ALL inputs host-prepped: bf16, pre-transposed/rolled, laid out for direct
  contiguous DMA into resident SBUF tiles (no on-chip casts or staging).
- Softmax: scores PSUM drained by Scalar exp directly (2-bank chunks), then
  one 2x-rate bf16 stt on Vector/GpSimd computes p'' = mask*(exp(s)-1).
  The mask correction  sum_k V'[k]  is a rank-1 matmul accumulated into the
  PV psum (colsum trick), so masked positions contribute exactly 1 (= exp(eps)).
- Z via ones-column appended to V'; 1/Z broadcast across partitions with a
  rank-1 matmul (no DRAM round-trips anywhere).
- Biases: per-partition ones via activation bias (bq,bk,b1,t2); free-axis ones
  via rank-1 matmuls into PSUM (bo, bv, b2); LN affine folded into x_aff
  tiles and host-precomputed rows (t1 -> tpos_t1 & r_ca; t2 -> r_ffn, t2pp).
- CA K/V' projections emitted interleaved with SA head loop (feeder) so the
  Tensor engine stays continuously busy (p-state).
"""

import sys

for _p in ("/opt/trn_rl_repo", "/root/.axon_site/_ro/trn_rl_repo"):
    if _p not in sys.path:
        sys.path.insert(0, _p)

import numpy as np

import concourse.bass as bass
import concourse.tile as tile
from concourse import bacc, mybir
from concourse.masks import make_identity

P = 128
D = 512
DC = D // P          # 4
S = 1024
SC = S // P          # 8
R = 512              # local query rows per core
RC = R // P          # 4
NH = 8
DK = D // NH         # 64
H = 2048
HC = H // P          # 16
LN_EPS = 1e-5
ISQ = 1.0 / 8.0      # 1/sqrt(dk)

F32 = mybir.dt.float32
F32R = mybir.dt.float32r
BF16 = mybir.dt.bfloat16
AF = mybir.ActivationFunctionType
OP = mybir.AluOpType

# rows_d indices
ROW_SA, ROW_CA, ROW_FFN, ROW_BV_SA, ROW_BV_CA = range(5)
# reps (partition-broadcast) indices
REP_G1, REP_G2, REP_G3, REP_T3 = range(4)

_ACT_FUNCS = None


def _patch_act_tables():
    """Strip our activation funcs from every table set except the one set that
    covers them all, so bacc emits a single ACT_TABLE_LOAD (ids unchanged)."""
    global _ACT_FUNCS
    if _ACT_FUNCS is not None:
        return
    from concourse.hw_specs import get_activation_tables as _gat

    mine = {AF.Exp, AF.Ln, AF.Relu, AF.Identity, AF.Copy}

    def patched(arch):
        t = _gat(arch)
        return {
            name: (s if name == "natural_log_exp_and_others" else (s - mine))
            for name, s in t.items()
        }

    bacc.get_activation_tables = patched
    _ACT_FUNCS = mine


def build_nc():
    _patch_act_tables()
    nc = bacc.Bacc()
    d = {}
    for nm, shp, dt in (
        [("xq_sa", [P, DC, S], BF16), ("xv_sa", [P, DC, S], BF16),
         ("xq_ca", [P, DC, S], BF16), ("xv_ca", [P, DC, S], BF16),
         ("res0", [P, RC, D], BF16), ("tpos_t1", [P, DC, R], BF16),
         ("m_sa", [P, SC, R], BF16), ("m_ca", [P, SC, R], BF16),
         ("w1", [P, DC, H], BF16), ("w2", [P, HC, D], BF16),
         ("bq_sa", [P, DC], F32), ("bk_sa", [P, DC], F32),
         ("bq_ca", [P, DC], F32), ("bk_ca", [P, DC], F32),
         ("t2pp", [P, DC], F32), ("b1pp", [P, HC], F32),
         ("rows_d", [P, 5 * D], BF16), ("grep_d", [1, 4 * D], BF16)]
        + [(f"w_{pre}_{w}", [P, DC, D], BF16)
           for pre in ("sa", "ca") for w in "qkvo"]
    ):
        d[nm] = nc.dram_tensor(nm, shp, dt, kind="ExternalInput")
    out_d = nc.dram_tensor("out", [P, RC, D], F32, kind="ExternalOutput")
    with tile.TileContext(nc) as tc:
        _body(nc, tc, d, out_d)
    nc.finalize()
    return nc


def _body(nc, tc, d, out_d):
    import os
    from contextlib import ExitStack

    DBG = bool(os.environ.get("KDBG"))

    def dump(name, ap, dtype):
        if not DBG:
            return
        dt_ = nc.dram_tensor(name, list(ap.shape), dtype, kind="ExternalOutput")
        nc.sync.dma_start(dt_[tuple(slice(None) for _ in ap.shape)], ap)

    with ExitStack() as ctx:
        const = ctx.enter_context(tc.tile_pool(name="const", bufs=1))
        wpool = ctx.enter_context(tc.tile_pool(name="w", bufs=1))
        xt = ctx.enter_context(tc.tile_pool(name="xt", bufs=2))
        mp = ctx.enter_context(tc.tile_pool(name="mp", bufs=2))
        qtp = ctx.enter_context(tc.tile_pool(name="qtp", bufs=1))
        ktp = ctx.enter_context(tc.tile_pool(name="ktp", bufs=2))
        vpp = ctx.enter_context(tc.tile_pool(name="vpp", bufs=2))
        spp = ctx.enter_context(tc.tile_pool(name="spp", bufs=2))
        atp = ctx.enter_context(tc.tile_pool(name="atp", bufs=1))
        xnp = ctx.enter_context(tc.tile_pool(name="xnp", bufs=1))
        hcp = ctx.enter_context(tc.tile_pool(name="hcp", bufs=2))
        stp = ctx.enter_context(tc.tile_pool(name="stp", bufs=2))
        sml = ctx.enter_context(tc.tile_pool(name="sml", bufs=2))
        # PSUM: mm(2) + sc(2x2banks) + pv(2) = 8 banks
        psm = ctx.enter_context(tc.tile_pool(name="psm", bufs=2, space="PSUM"))
        pss = ctx.enter_context(tc.tile_pool(name="pss", bufs=2, space="PSUM"))
        psv = ctx.enter_context(tc.tile_pool(name="psv", bufs=2, space="PSUM"))

        # ---------------- consts ----------------
        ident = const.tile([P, P], BF16)
        make_identity(nc, ident)
        eps_t = const.tile([P, 1], F32)
        nc.vector.memset(eps_t, LN_EPS)
        ones_row = const.tile([P, D], BF16)
        nc.vector.memset(ones_row, 0.0)
        nc.vector.memset(ones_row[0:1, :], 1.0)
        rzb = const.tile([P, R], BF16)
        nc.vector.memset(rzb, 0.0)
        ones_col = const.tile([P, 1], BF16)
        nc.vector.memset(ones_col, 1.0)

        # ---------------- input DMAs (issued up front) ----------------
        # sync (SP HWDGE): SA-critical stream
        sa_w = {}
        ca_w = {}
        xq1 = xt.tile([P, DC, S], BF16, tag="xq", name="xq1")
        nc.sync.dma_start(xq1[:, :, 0:R], d["xq_sa"][:, :, 0:R])
        for w in "qk":
            sa_w[w] = wpool.tile([P, DC, D], BF16, tag="wa", bufs=8, name=f"wsa{w}")
            nc.scalar.dma_start(sa_w[w], d[f"w_sa_{w}"][:, :, :])
        nc.sync.dma_start(xq1[:, :, R:S], d["xq_sa"][:, :, R:S])
        bq_sa = const.tile([P, DC], F32)
        nc.sync.dma_start(bq_sa, d["bq_sa"][:, :])
        bk_sa = const.tile([P, DC], F32)
        nc.sync.dma_start(bk_sa, d["bk_sa"][:, :])
        xv1 = xt.tile([P, DC, S], BF16, tag="xv", name="xv1")
        nc.sync.dma_start(xv1, d["xv_sa"][:, :, :])
        sa_w["v"] = wpool.tile([P, DC, D], BF16, tag="wa", bufs=8, name="wsav")
        nc.sync.dma_start(sa_w["v"], d["w_sa_v"][:, :, :])
        rows_sb = const.tile([P, 5, D], BF16)
        nc.sync.dma_start(rows_sb, d["rows_d"][:, :].rearrange("a (r n) -> a r n", n=D))
        msa = mp.tile([P, SC, R], BF16, tag="m", name="msa")
        nc.gpsimd.dma_start(msa, d["m_sa"][:, :, :])
        sa_w["o"] = wpool.tile([P, DC, D], BF16, tag="wa", bufs=8, name="wsao")
        nc.sync.dma_start(sa_w["o"], d["w_sa_o"][:, :, :])
        res0 = const.tile([P, RC, D], BF16)
        nc.sync.dma_start(res0, d["res0"][:, :, :])

        # scalar (Act HWDGE): CA stream (issued before any activation work)
        xq2 = xt.tile([P, DC, S], BF16, tag="xq", name="xq2")
        nc.scalar.dma_start(xq2, d["xq_ca"][:, :, :])
        for w in "kv":
            ca_w[w] = wpool.tile([P, DC, D], BF16, tag="wa", bufs=8, name=f"wca{w}")
            nc.scalar.dma_start(ca_w[w], d[f"w_ca_{w}"][:, :, :])
        bk_ca = const.tile([P, DC], F32)
        nc.scalar.dma_start(bk_ca, d["bk_ca"][:, :])
        xv2 = xt.tile([P, DC, S], BF16, tag="xv", name="xv2")
        nc.scalar.dma_start(xv2, d["xv_ca"][:, :, :])
        mca = mp.tile([P, SC, R], BF16, tag="m", name="mca")
        nc.gpsimd.dma_start(mca, d["m_ca"][:, :, :])
        for w in "qo":
            ca_w[w] = wpool.tile([P, DC, D], BF16, tag="wa", bufs=8, name=f"wca{w}")
            nc.scalar.dma_start(ca_w[w], d[f"w_ca_{w}"][:, :, :])
        bq_ca = const.tile([P, DC], F32)
        nc.scalar.dma_start(bq_ca, d["bq_ca"][:, :])
        tpos_t1 = const.tile([P, DC, R], BF16)
        nc.scalar.dma_start(tpos_t1, d["tpos_t1"][:, :, :])

        # FFN stream on sync (after SA-critical loads); reps broadcast on gpsimd
        w1b = wpool.tile([P, DC, H], BF16, tag="w1")
        nc.sync.dma_start(w1b, d["w1"][:, :, :])
        b1pp = const.tile([P, HC], F32)
        nc.sync.dma_start(b1pp, d["b1pp"][:, :])
        w2b = wpool.tile([P, HC, D], BF16, tag="w2")
        nc.sync.dma_start(w2b, d["w2"][:, :, :])
        t2pp = const.tile([P, DC], F32)
        nc.sync.dma_start(t2pp, d["t2pp"][:, :])
        reps = const.tile([P, 4, D], BF16)
        nc.gpsimd.dma_start(reps, d["grep_d"][0, :].partition_broadcast(P))

        # ---------------- helpers ----------------
        def qproj(xqT, wq, bq_pp, name):
            """Q^T [P, DC, R] bf16 (bias via activation; 1/sqrt(dk) folded)."""
            qT = qtp.tile([P, DC, R], BF16, tag="qT", name=name)
            for mo in range(DC):
                ps = psm.tile([P, R], F32, tag="mm", name="qps")
                for kc in range(DC):
                    nc.tensor.matmul(
                        ps, lhsT=wq[:, kc, mo * P:(mo + 1) * P],
                        rhs=xqT[:, kc, 0:R],
                        start=(kc == 0), stop=(kc == DC - 1),
                    )
                nc.scalar.activation(
                    qT[:, mo, :], ps, AF.Identity, bias=bq_pp[:, mo:mo + 1]
                )
            return qT

        def kproj_groups(xkT, wk, bk_pp, kT):
            """Returns 8 thunks, each emitting one K^T chunk [P, mo, ns*R:]."""
            def mk(mo, ns):
                def g():
                    ps = psm.tile([P, R], F32, tag="mm", name="kps")
                    for kc in range(DC):
                        nc.tensor.matmul(
                            ps, lhsT=wk[:, kc, mo * P:(mo + 1) * P],
                            rhs=xkT[:, kc, ns * R:(ns + 1) * R],
                            start=(kc == 0), stop=(kc == DC - 1),
                        )
                    nc.scalar.activation(
                        kT[:, mo, ns * R:(ns + 1) * R], ps, AF.Identity,
                        bias=bk_pp[:, mo:mo + 1],
                    )
                return g
            return [mk(mo, ns) for mo in range(DC) for ns in range(2)]

        def vproj_groups(xvT, wv, bv_row, vp):
            """Returns 1 memset + 8 V' chunk thunks into vp [P, SC, NH, DK+1]."""
            def ms():
                nc.vector.memset(vp[:, :, :, DK:DK + 1], 1.0)
            def mk(rc):
                def g():
                    ps = psm.tile([P, D], F32, tag="mm", name="vps")
                    for kc in range(DC):
                        nc.tensor.matmul(
                            ps, lhsT=xvT[:, kc, rc * P:(rc + 1) * P],
                            rhs=wv[:, kc, :],
                            start=(kc == 0), stop=False,
                        )
                    nc.tensor.matmul(
                        ps, lhsT=ones_row[:, 0:P], rhs=bv_row,
                        start=False, stop=True,
                    )
                    nc.scalar.copy(vp[:, rc, :, 0:DK], ps)
                return g
            return [ms] + [mk(rc) for rc in range(SC)]

        def colsum_groups(vp, csS):
            """3 thunks: colsum [32, NH, DK+1]; row 0 = sum_k V', rows 1-31 = 0
            (K=1 matmul operands must be zero-padded to the 32-row PE tile)."""
            def ms():
                nc.vector.memset(csS, 0.0)
            def mk(i):
                def g():
                    ps = psm.tile([1, 4 * (DK + 1)], F32, tag="mm", name="csps")
                    for kc in range(SC):
                        nc.tensor.matmul(
                            ps, lhsT=ones_col,
                            rhs=vp[:, kc, 4 * i:4 * (i + 1), :],
                            start=(kc == 0), stop=(kc == SC - 1),
                        )
                    nc.scalar.copy(csS[0:1, 4 * i:4 * (i + 1), :], ps)
                return g
            return [ms, mk(0), mk(1)]

        def head(h, qT, kT, vp, msk, csS, attnT, tag):
            """One attention head: scores -> exp -> mask -> PV(+corr) -> norm."""
            po, ch = (h % 2) * DK, h // 2
            sp = spp.tile([P, SC, R], BF16, tag="sp", name=f"sp_{tag}{h}")
            for jj in range(4):
                sc = pss.tile([P, 2, R], F32, tag="sc", name="scps")
                for i in range(2):
                    kc = 2 * jj + i
                    nc.tensor.matmul(
                        sc[:, i, :],
                        lhsT=kT[po:po + DK, ch, kc * P:(kc + 1) * P],
                        rhs=qT[po:po + DK, ch, :], start=True, stop=True,
                    )
                nc.scalar.activation(sp[:, 2 * jj:2 * jj + 2, :], sc[:], AF.Exp)
                if jj % 2 == 1:
                    half = jj // 2
                    nc.vector.scalar_tensor_tensor(
                        out=sp[:, 4 * half:4 * half + 4, :],
                        in0=sp[:, 4 * half:4 * half + 4, :], scalar=1.0,
                        in1=msk[:, 4 * half:4 * half + 4, :],
                        op0=OP.subtract, op1=OP.mult,
                    )
            if DBG and h == 0:
                dump(f"dbg_sp0_{tag}", sp[:], BF16)
            pv = psv.tile([P, R], F32, tag="pv", name="pvps")
            for kc in range(SC):
                nc.tensor.matmul(
                    pv[0:DK + 1, :], lhsT=vp[:, kc, h, :], rhs=sp[:, kc, :],
                    start=(kc == 0), stop=False,
                )
            nc.tensor.matmul(
                pv[0:DK + 1, :], lhsT=csS[:, h, :], rhs=ones_row[:, 0:R],
                start=False, stop=True,
            )
            rz = sml.tile([1, R], F32, tag="rz", bufs=1, name=f"rz_{tag}{h}")
            nc.vector.tensor_copy(out=rz, in_=pv[DK:DK + 1, :])
            nc.vector.reciprocal_approx_fast(out=rz, in_=rz)
            if DBG and h == 0:
                dump(f"dbg_rz0_{tag}", rz[:], F32)
            nc.vector.tensor_copy(out=rzb[0:1, :], in_=rz)
            rr = psm.tile([P, R], F32, tag="mm", name="rrps")
            nc.tensor.matmul(
                rr[0:DK, :], lhsT=ones_row[:, 0:DK], rhs=rzb,
                start=True, stop=True,
            )
            rrS = sml.tile([DK, R], BF16, tag="rrS", bufs=1, name=f"rrS_{tag}{h}")
            nc.scalar.copy(rrS, rr[0:DK, :])
            if DBG and h == 0:
                dump(f"dbg_rrS0_{tag}", rrS[:], BF16)
            nc.vector.tensor_mul(attnT[po:po + DK, ch, :], pv[0:DK, :], rrS)

        def outproj(attnT, wo, row_i, resid, xpre_name):
            """xpre [P, RC, D] bf16 = attn @ wo + row + resid."""
            xpre = xnp.tile([P, RC, D], BF16, tag="xpre", name=xpre_name)
            for qm in range(RC):
                ps = psm.tile([P, D], F32, tag="mm", name="ops")
                for kc in range(DC):
                    nc.tensor.matmul(
                        ps, lhsT=attnT[:, kc, qm * P:(qm + 1) * P],
                        rhs=wo[:, kc, :],
                        start=(kc == 0), stop=False,
                    )
                nc.tensor.matmul(
                    ps, lhsT=ones_row[:, 0:P],
                    rhs=rows_sb[:, row_i, :], start=False, stop=True,
                )
                nc.vector.tensor_add(out=xpre[:, qm, :], in0=ps,
                                     in1=resid[:, qm, :])
            return xpre

        def layernorm(xpre, name):
            """In-place: xpre <- (x - mu) * rsqrt(var + eps)   (no affine)."""
            y = xpre
            for qm in range(RC):
                stats = stp.tile([P, 6], F32, tag="stats")
                nc.vector.bn_stats(stats, xpre[:, qm, :])
                mv = stp.tile([P, 2], F32, tag="mv")
                nc.vector.bn_aggr(mv, stats)
                t = stp.tile([P, 1], F32, tag="ln_t")
                nc.scalar.activation(t, mv[:, 1:2], AF.Ln, bias=eps_t)
                rstd = stp.tile([P, 1], F32, tag="ln_rstd")
                nc.scalar.activation(rstd, t, AF.Exp, scale=-0.5)
                nc.vector.tensor_scalar(
                    out=y[:, qm, :], in0=xpre[:, qm, :],
                    scalar1=mv[:, 0:1], scalar2=rstd,
                    op0=OP.subtract, op1=OP.mult,
                )
            return y

        def affine(y, rep_i, name):
            """x_aff = y * g_rep  (bf16, one 2x stt per row-block)."""
            aff = xnp.tile([P, RC, D], BF16, tag="aff", name=name)
            for qm in range(RC):
                nc.vector.tensor_mul(aff[:, qm, :], y[:, qm, :],
                                     reps[:, rep_i, :])
            return aff

        def transpose_nat(x_nat, drain, name):
            """[P, RC, D] natural -> [P, DC, R] transposed via identity mms.
            drain(c, psum_ap) writes channel block c."""
            xT_ = xnp.tile([P, DC, R], BF16, tag="xT", name=name)
            for c in range(DC):
                pt = psm.tile([P, R], F32, tag="mm", name="tps")
                for qm in range(RC):
                    nc.tensor.matmul(
                        pt[:, qm * P:(qm + 1) * P],
                        lhsT=x_nat[:, qm, c * P:(c + 1) * P], rhs=ident,
                        start=(qm == 0), stop=(qm == RC - 1),
                    )
                drain(c, pt, xT_)
            return xT_

        # ================= SA =================
        qT1 = qproj(xq1, sa_w["q"], bq_sa, "qT1")
        kT1 = ktp.tile([P, DC, S], BF16, tag="kT", name="kT1")
        for g in kproj_groups(xq1, sa_w["k"], bk_sa, kT1):
            g()
        vp1 = vpp.tile([P, SC, NH, DK + 1], BF16, tag="vp", name="vp1")
        for g in vproj_groups(xv1, sa_w["v"], rows_sb[:, ROW_BV_SA, :], vp1):
            g()
        cs1 = sml.tile([P, NH, DK + 1], BF16, tag="cs", name="cs1")
        for g in colsum_groups(vp1, cs1):
            g()

        # CA prefetch feeder: emitted interleaved with SA heads
        kT2 = ktp.tile([P, DC, S], BF16, tag="kT", name="kT2")
        vp2 = vpp.tile([P, SC, NH, DK + 1], BF16, tag="vp", name="vp2")
        cs2 = sml.tile([P, NH, DK + 1], BF16, tag="cs", name="cs2")
        feeder = (
            kproj_groups(xq2, ca_w["k"], bk_ca, kT2)
            + vproj_groups(xv2, ca_w["v"], rows_sb[:, ROW_BV_CA, :], vp2)
            + colsum_groups(vp2, cs2)
        )

        attnT1 = atp.tile([P, DC, R], BF16, tag="attnT", name="attnT1")
        for h in range(NH):
            head(h, qT1, kT1, vp1, msa, cs1, attnT1, "sa")
            for _ in range(3):
                if feeder:
                    feeder.pop(0)()
        while feeder:
            feeder.pop(0)()

        if DBG:
            dump("dbg_qT1", qT1[:], BF16)
            dump("dbg_kT1", kT1[:], BF16)
            dump("dbg_vp1", vp1[:], BF16)
            dump("dbg_cs1", cs1[:], BF16)
            dump("dbg_attnT1", attnT1[:], BF16)

        xpre1 = outproj(attnT1, sa_w["o"], ROW_SA, res0, "xpre1")
        y1 = layernorm(xpre1, "y1")
        x1aff = affine(y1, REP_G1, "x1aff")

        def drain_x1q(c, pt, xT_):
            nc.vector.tensor_add(out=xT_[:, c, :], in0=pt,
                                 in1=tpos_t1[:, c, :])
        x1qT = transpose_nat(x1aff, drain_x1q, "x1qT")

        if DBG:
            dump("dbg_xpre1", xpre1[:], BF16)
            dump("dbg_y1", y1[:], BF16)
            dump("dbg_x1aff", x1aff[:], BF16)
            dump("dbg_x1qT", x1qT[:], BF16)

        # ================= CA =================
        qT2 = qproj(x1qT, ca_w["q"], bq_ca, "qT2")
        attnT2 = atp.tile([P, DC, R], BF16, tag="attnT", name="attnT2")
        for h in range(NH):
            head(h, qT2, kT2, vp2, mca, cs2, attnT2, "ca")

        xpre2 = outproj(attnT2, ca_w["o"], ROW_CA, x1aff, "xpre2")
        y2 = layernorm(xpre2, "y2")
        x2aff = affine(y2, REP_G2, "x2aff")

        def drain_x2T(c, pt, xT_):
            nc.scalar.activation(xT_[:, c, :], pt, AF.Identity,
                                 bias=t2pp[:, c:c + 1])
        x2T = transpose_nat(x2aff, drain_x2T, "x2T")

        if DBG:
            dump("dbg_xpre2", xpre2[:], BF16)
            dump("dbg_x2aff", x2aff[:], BF16)
            dump("dbg_x2T", x2T[:], BF16)

        # ================= FFN =================
        psf = []
        for i in range(2):
            pf = pss.tile([P, 2, R], F32, tag="sc", name=f"psf{i}")
            psf.append(pf)

        def psf_ap(qm):
            return psf[qm // 2][:, qm % 2, :]

        for hc in range(HC):
            ph = psm.tile([P, R], F32, tag="mm", name="phps")
            for kc in range(DC):
                nc.tensor.matmul(
                    ph, lhsT=w1b[:, kc, hc * P:(hc + 1) * P], rhs=x2T[:, kc, :],
                    start=(kc == 0), stop=(kc == DC - 1),
                )
            hcb = hcp.tile([P, R], BF16, tag="hc")
            nc.scalar.activation(hcb, ph, AF.Relu, bias=b1pp[:, hc:hc + 1])
            for qm in range(RC):
                nc.tensor.matmul(
                    psf_ap(qm), lhsT=hcb[:, qm * P:(qm + 1) * P],
                    rhs=w2b[:, hc, :],
                    start=(hc == 0), stop=False,
                )
        xpre3 = xnp.tile([P, RC, D], BF16, tag="xpre", name="xpre3")
        for qm in range(RC):
            nc.tensor.matmul(
                psf_ap(qm), lhsT=ones_row[:, 0:P],
                rhs=rows_sb[:, ROW_FFN, :], start=False, stop=True,
            )
            nc.vector.tensor_add(out=xpre3[:, qm, :], in0=psf_ap(qm),
                                 in1=x2aff[:, qm, :])
        if DBG:
            dump("dbg_xpre3", xpre3[:], BF16)

        y3 = layernorm(xpre3, "y3")
        for qm in range(RC):
            ot = atp.tile([P, D], F32, tag="attnT", name="ot")
            nc.vector.tensor_mul(ot, y3[:, qm, :], reps[:, REP_G3, :])
            nc.vector.tensor_add(out=ot, in0=ot, in1=reps[:, REP_T3, :])
            nc.sync.dma_start(out_d[:, qm, :], ot)


# ----------------------------------------------------------------------------
_NC_CACHE = None


def _get_nc():
    global _NC_CACHE
    if _NC_CACHE is None:
        _NC_CACHE = build_nc()
    return _NC_CACHE


def _layP(a, nchunk):
    """[nchunk*P, ...] -> [P, nchunk, ...] contiguous."""
    a = a.reshape(nchunk, P, *a.shape[1:])
    return np.ascontiguousarray(a.transpose(1, 0, *range(2, a.ndim)))


def make_in_maps(inputs):
    """Shard + preprocess full inputs -> 8 per-core in_maps (all bf16-ready)."""
    import ml_dtypes

    bf = ml_dtypes.bfloat16
    t = {k: np.asarray(v, dtype=np.float32) if np.asarray(v).dtype != np.int32
         else np.asarray(v) for k, v in inputs.items()}
    in_maps = []
    for core in range(8):
        b, half = core // 2, core % 2
        rows = slice(half * R, half * R + R)
        xq_sa = np.roll((t["tgt"][b] + t["tgt_pos"][b]).T, -half * R, axis=1)
        xv_sa = np.roll(t["tgt"][b].T, -half * R, axis=1)
        xq_ca = (t["memory"][b] + t["memory_pos"][b]).T
        xv_ca = t["memory"][b].T
        m_sa = np.roll(t["tgt_mask"][b, rows, :], -half * R, axis=1).T
        m_ca = t["memory_mask"][b, rows, :].T
        m = {
            "xq_sa": _layP(xq_sa.astype(bf), DC),
            "xv_sa": _layP(xv_sa.astype(bf), DC),
            "xq_ca": _layP(xq_ca.astype(bf), DC),
            "xv_ca": _layP(xv_ca.astype(bf), DC),
            "res0": _layP(t["tgt"][b, rows, :].astype(bf), RC),
            "tpos_t1": _layP(
                (t["tgt_pos"][b, rows, :].T + t["ln1_b"][:, None]).astype(bf), DC),
            "m_sa": _layP(m_sa.astype(np.float32).astype(bf), SC),
            "m_ca": _layP(m_ca.astype(np.float32).astype(bf), SC),
            "w1": _layP(t["mlp_w1"].astype(bf), DC),
            "w2": _layP(t["mlp_w2"].astype(bf), HC),
            "t2pp": np.ascontiguousarray(t["ln2_b"].reshape(DC, P).T),
            "b1pp": np.ascontiguousarray(t["mlp_b1"].reshape(HC, P).T),
            "rows_d": np.concatenate([
                np.concatenate([
                    t["sa_bo"],
                    t["ca_bo"] + t["ln1_b"],
                    t["mlp_b2"] + t["ln2_b"],
                    t["sa_bv"],
                    t["ca_bv"],
                ])[None, :],
                np.zeros((127, 5 * D), np.float32),
            ]).astype(bf),
            "grep_d": np.concatenate([
                t["ln1_g"], t["ln2_g"], t["ln3_g"], t["ln3_b"],
            ])[None, :].astype(bf),
        }
        for pre in ("sa", "ca"):
            for w in "qkvo":
                scl = ISQ if w == "q" else 1.0
                m[f"w_{pre}_{w}"] = _layP((t[f"{pre}_w{w}"] * scl).astype(bf), DC)
            m[f"bq_{pre}"] = np.ascontiguousarray(
                (t[f"{pre}_bq"] * ISQ).reshape(DC, P).T.astype(np.float32))
            m[f"bk_{pre}"] = np.ascontiguousarray(
                t[f"{pre}_bk"].reshape(DC, P).T.astype(np.float32))
        in_maps.append(m)
    return in_maps


def gather_out(results):
    out = np.zeros((4, S, D), np.float32)
    for core in range(8):
        b, half = core // 2, core % 2
        o = np.asarray(results[core]["out"], dtype=np.float32)  # [P, RC, D]
        out[b, half * R:half * R + R, :] = o.transpose(1, 0, 2).reshape(R, D)
    return out


def kernel(**inputs):
    from concourse import bass_utils

    nc = _get_nc()
    in_maps = make_in_maps(inputs)
    res = bass_utils.run_bass_kernel_spmd(nc, in_maps, core_ids=list(range(8)))
    return gather_out(res.results)


# revision 16
# speedup vs baseline: 1.0855x; 1.0855x over previous
"""Trainium2 Bass kernel v2 for nn_CrossAttnLayer (post-LN decoder layer:
self-attn + cross-attn + FFN).  B=4, S=M=1024, D=512, nhead=8, H=2048.

Sharding: 8 cores = (batch b = core//2) x (query-row half = core%2); each core
computes R=512 query rows of one batch end-to-end.  No collectives.

v2 design (vs baseline):
- ALL inputs host-prepped: bf16, pre-transposed/rolled, laid out for direct
  contiguous DMA into resident SBUF tiles (no on-chip casts or staging).
- Softmax: scores PSUM drained by Scalar exp directly (2-bank chunks), then
  one 2x-rate bf16 stt on Vector/GpSimd computes p'' = mask*(exp(s)-1).
  The mask correction  sum_k V'[k]  is a rank-1 matmul accumulated into the
  PV psum (colsum trick), so masked positions contribute exactly 1 (= exp(eps)).
- Z via ones-column appended to V'; 1/Z broadcast across partitions with a
  rank-1 matmul (no DRAM round-trips anywhere).
- Biases: per-partition ones via activation bias (bq,bk,b1,t2); free-axis ones
  via rank-1 matmuls into PSUM (bo, bv, b2); LN affine folded into x_aff
  tiles and host-precomputed rows (t1 -> tpos_t1 & r_ca; t2 -> r_ffn, t2pp).
- CA K/V' projections emitted interleaved with SA head loop (feeder) so the
  Tensor engine stays continuously busy (p-state).
"""

import sys

for _p in ("/opt/trn_rl_repo", "/root/.axon_site/_ro/trn_rl_repo"):
    if _p not in sys.path:
        sys.path.insert(0, _p)

import numpy as np

import concourse.bass as bass
import concourse.tile as tile
from concourse import bacc, mybir
from concourse.masks import make_identity

P = 128
D = 512
DC = D // P          # 4
S = 1024
SC = S // P          # 8
R = 512              # local query rows per core
RC = R // P          # 4
NH = 8
DK = D // NH         # 64
H = 2048
HC = H // P          # 16
LN_EPS = 1e-5
ISQ = 1.0 / 8.0      # 1/sqrt(dk)

F32 = mybir.dt.float32
F32R = mybir.dt.float32r
BF16 = mybir.dt.bfloat16
AF = mybir.ActivationFunctionType
OP = mybir.AluOpType

# rows_d indices
ROW_SA, ROW_CA, ROW_FFN, ROW_BV_SA, ROW_BV_CA = range(5)
# reps (partition-broadcast) indices
REP_G1, REP_G2, REP_G3, REP_T3 = range(4)

_ACT_FUNCS = None


def _patch_act_tables():
    """Strip our activation funcs from every table set except the one set that
    covers them all, so bacc emits a single ACT_TABLE_LOAD (ids unchanged)."""
    global _ACT_FUNCS
    if _ACT_FUNCS is not None:
        return
    from concourse.hw_specs import get_activation_tables as _gat

    mine = {AF.Exp, AF.Ln, AF.Relu, AF.Identity, AF.Copy}

    def patched(arch):
        t = _gat(arch)
        return {
            name: (s if name == "natural_log_exp_and_others" else (s - mine))
            for name, s in t.items()
        }

    bacc.get_activation_tables = patched
    _ACT_FUNCS = mine


def build_nc():
    _patch_act_tables()
    nc = bacc.Bacc()
    d = {}
    for nm, shp, dt in (
        [("xq_sa", [P, DC, S], BF16), ("xv_sa", [P, DC, S], BF16),
         ("xq_ca", [P, DC, S], BF16), ("xv_ca", [P, DC, S], BF16),
         ("res0", [P, RC, D], BF16), ("tpos_t1", [P, DC, R], BF16),
         ("m_sa", [P, SC, R], BF16), ("m_ca", [P, SC, R], BF16),
         ("w1", [P, DC, H], BF16), ("w2", [P, HC, D], BF16),
         ("bq_sa", [P, DC], F32), ("bk_sa", [P, DC], F32),
         ("bq_ca", [P, DC], F32), ("bk_ca", [P, DC], F32),
         ("t2pp", [P, DC], F32), ("b1pp", [P, HC], F32),
         ("rows_d", [P, 5 * D], BF16), ("grep_d", [1, 4 * D], BF16)]
        + [(f"w_{pre}_{w}", [P, DC, D], BF16)
           for pre in ("sa", "ca") for w in "qkvo"]
    ):
        d[nm] = nc.dram_tensor(nm, shp, dt, kind="ExternalInput")
    out_d = nc.dram_tensor("out", [P, RC, D], F32, kind="ExternalOutput")
    with tile.TileContext(nc) as tc:
        _body(nc, tc, d, out_d)
    nc.finalize()
    return nc


def _body(nc, tc, d, out_d):
    import os
    from contextlib import ExitStack

    DBG = bool(os.environ.get("KDBG"))

    def dump(name, ap, dtype):
        if not DBG:
            return
        dt_ = nc.dram_tensor(name, list(ap.shape), dtype, kind="ExternalOutput")
        nc.sync.dma_start(dt_[tuple(slice(None) for _ in ap.shape)], ap)

    with ExitStack() as ctx:
        const = ctx.enter_context(tc.tile_pool(name="const", bufs=1))
        wpool = ctx.enter_context(tc.tile_pool(name="w", bufs=1))
        xt = ctx.enter_context(tc.tile_pool(name="xt", bufs=2))
        mp = ctx.enter_context(tc.tile_pool(name="mp", bufs=2))
        qtp = ctx.enter_context(tc.tile_pool(name="qtp", bufs=1))
        ktp = ctx.enter_context(tc.tile_pool(name="ktp", bufs=2))
        vpp = ctx.enter_context(tc.tile_pool(name="vpp", bufs=2))
        spp = ctx.enter_context(tc.tile_pool(name="spp", bufs=2))
        atp = ctx.enter_context(tc.tile_pool(name="atp", bufs=1))
        xnp = ctx.enter_context(tc.tile_pool(name="xnp", bufs=1))
        hcp = ctx.enter_context(tc.tile_pool(name="hcp", bufs=2))
        stp = ctx.enter_context(tc.tile_pool(name="stp", bufs=2))
        sml = ctx.enter_context(tc.tile_pool(name="sml", bufs=2))
        # PSUM: mm(2) + sc(2x2banks) + pv(2) = 8 banks
        psm = ctx.enter_context(tc.tile_pool(name="psm", bufs=2, space="PSUM"))
        pss = ctx.enter_context(tc.tile_pool(name="pss", bufs=2, space="PSUM"))
        psv = ctx.enter_context(tc.tile_pool(name="psv", bufs=2, space="PSUM"))

        # ---------------- consts ----------------
        ident = const.tile([P, P], BF16)
        make_identity(nc, ident)
        eps_t = const.tile([P, 1], F32)
        nc.vector.memset(eps_t, LN_EPS)
        ones_row = const.tile([P, D], BF16)
        nc.vector.memset(ones_row, 0.0)
        nc.vector.memset(ones_row[0:1, :], 1.0)
        rzb = const.tile([P, R], BF16)
        nc.vector.memset(rzb, 0.0)
        ones_col = const.tile([P, 1], BF16)
        nc.vector.memset(ones_col, 1.0)

        # ---------------- input DMAs (issued up front) ----------------
        # sync (SP HWDGE): SA-critical stream
        sa_w = {}
        ca_w = {}
        xq1 = xt.tile([P, DC, S], BF16, tag="xq", name="xq1")
        nc.sync.dma_start(xq1[:, :, 0:R], d["xq_sa"][:, :, 0:R])
        for w in "qk":
            sa_w[w] = wpool.tile([P, DC, D], BF16, tag="wa", bufs=8, name=f"wsa{w}")
            nc.scalar.dma_start(sa_w[w], d[f"w_sa_{w}"][:, :, :])
        nc.sync.dma_start(xq1[:, :, R:S], d["xq_sa"][:, :, R:S])
        bq_sa = const.tile([P, DC], F32)
        nc.sync.dma_start(bq_sa, d["bq_sa"][:, :])
        bk_sa = const.tile([P, DC], F32)
        nc.sync.dma_start(bk_sa, d["bk_sa"][:, :])
        xv1 = xt.tile([P, DC, S], BF16, tag="xv", name="xv1")
        nc.sync.dma_start(xv1, d["xv_sa"][:, :, :])
        sa_w["v"] = wpool.tile([P, DC, D], BF16, tag="wa", bufs=8, name="wsav")
        nc.sync.dma_start(sa_w["v"], d["w_sa_v"][:, :, :])
        rows_sb = const.tile([P, 5, D], BF16)
        nc.sync.dma_start(rows_sb, d["rows_d"][:, :].rearrange("a (r n) -> a r n", n=D))
        msa = mp.tile([P, SC, R], BF16, tag="m", name="msa")
        nc.gpsimd.dma_start(msa, d["m_sa"][:, :, :])
        sa_w["o"] = wpool.tile([P, DC, D], BF16, tag="wa", bufs=8, name="wsao")
        nc.sync.dma_start(sa_w["o"], d["w_sa_o"][:, :, :])
        res0 = const.tile([P, RC, D], BF16)
        nc.sync.dma_start(res0, d["res0"][:, :, :])

        # scalar (Act HWDGE): CA stream (issued before any activation work)
        xq2 = xt.tile([P, DC, S], BF16, tag="xq", name="xq2")
        nc.scalar.dma_start(xq2, d["xq_ca"][:, :, :])
        for w in "kv":
            ca_w[w] = wpool.tile([P, DC, D], BF16, tag="wa", bufs=8, name=f"wca{w}")
            nc.scalar.dma_start(ca_w[w], d[f"w_ca_{w}"][:, :, :])
        bk_ca = const.tile([P, DC], F32)
        nc.scalar.dma_start(bk_ca, d["bk_ca"][:, :])
        xv2 = xt.tile([P, DC, S], BF16, tag="xv", name="xv2")
        nc.scalar.dma_start(xv2, d["xv_ca"][:, :, :])
        mca = mp.tile([P, SC, R], BF16, tag="m", name="mca")
        nc.gpsimd.dma_start(mca, d["m_ca"][:, :, :])
        for w in "qo":
            ca_w[w] = wpool.tile([P, DC, D], BF16, tag="wa", bufs=8, name=f"wca{w}")
            nc.scalar.dma_start(ca_w[w], d[f"w_ca_{w}"][:, :, :])
        bq_ca = const.tile([P, DC], F32)
        nc.scalar.dma_start(bq_ca, d["bq_ca"][:, :])
        tpos_t1 = const.tile([P, DC, R], BF16)
        nc.scalar.dma_start(tpos_t1, d["tpos_t1"][:, :, :])

        # FFN stream on sync (after SA-critical loads); reps broadcast on gpsimd
        w1b = wpool.tile([P, DC, H], BF16, tag="w1")
        nc.sync.dma_start(w1b, d["w1"][:, :, :])
        b1pp = const.tile([P, HC], F32)
        nc.sync.dma_start(b1pp, d["b1pp"][:, :])
        w2b = wpool.tile([P, HC, D], BF16, tag="w2")
        nc.sync.dma_start(w2b, d["w2"][:, :, :])
        t2pp = const.tile([P, DC], F32)
        nc.sync.dma_start(t2pp, d["t2pp"][:, :])
        reps = const.tile([P, 4, D], BF16)
        nc.gpsimd.dma_start(reps, d["grep_d"][0, :].partition_broadcast(P))

        # ---------------- helpers ----------------
        def qproj(xqT, wq, bq_pp, name):
            """Q^T [P, DC, R] bf16 (bias via activation; 1/sqrt(dk) folded)."""
            qT = qtp.tile([P, DC, R], BF16, tag="qT", name=name)
            for mo in range(DC):
                ps = psm.tile([P, R], F32, tag="mm", name="qps")
                for kc in range(DC):
                    nc.tensor.matmul(
                        ps, lhsT=wq[:, kc, mo * P:(mo + 1) * P],
                        rhs=xqT[:, kc, 0:R],
                        start=(kc == 0), stop=(kc == DC - 1),
                    )
                nc.scalar.activation(
                    qT[:, mo, :], ps, AF.Identity, bias=bq_pp[:, mo:mo + 1]
                )
            return qT

        def kproj_groups(xkT, wk, bk_pp, kT):
            """Returns 8 thunks, each emitting one K^T chunk [P, mo, ns*R:]."""
            def mk(mo, ns):
                def g():
                    ps = psm.tile([P, R], F32, tag="mm", name="kps")
                    for kc in range(DC):
                        nc.tensor.matmul(
                            ps, lhsT=wk[:, kc, mo * P:(mo + 1) * P],
                            rhs=xkT[:, kc, ns * R:(ns + 1) * R],
                            start=(kc == 0), stop=(kc == DC - 1),
                        )
                    nc.scalar.activation(
                        kT[:, mo, ns * R:(ns + 1) * R], ps, AF.Identity,
                        bias=bk_pp[:, mo:mo + 1],
                    )
                return g
            return [mk(mo, ns) for mo in range(DC) for ns in range(2)]

        def vproj_groups(xvT, wv, bv_row, vp):
            """Returns 1 memset + 8 V' chunk thunks into vp [P, SC, NH, DK+1]."""
            def ms():
                nc.vector.memset(vp[:, :, :, DK:DK + 1], 1.0)
            def mk(rc):
                def g():
                    ps = psm.tile([P, D], F32, tag="mm", name="vps")
                    for kc in range(DC):
                        nc.tensor.matmul(
                            ps, lhsT=xvT[:, kc, rc * P:(rc + 1) * P],
                            rhs=wv[:, kc, :],
                            start=(kc == 0), stop=False,
                        )
                    nc.tensor.matmul(
                        ps, lhsT=ones_row[:, 0:P], rhs=bv_row,
                        start=False, stop=True,
                    )
                    nc.scalar.copy(vp[:, rc, :, 0:DK], ps)
                return g
            return [ms] + [mk(rc) for rc in range(SC)]

        def colsum_groups(vp, csS):
            """3 thunks: colsum [32, NH, DK+1]; row 0 = sum_k V', rows 1-31 = 0
            (K=1 matmul operands must be zero-padded to the 32-row PE tile)."""
            def ms():
                nc.vector.memset(csS, 0.0)
            def mk(i):
                def g():
                    ps = psm.tile([1, 4 * (DK + 1)], F32, tag="mm", name="csps")
                    for kc in range(SC):
                        nc.tensor.matmul(
                            ps, lhsT=ones_col,
                            rhs=vp[:, kc, 4 * i:4 * (i + 1), :],
                            start=(kc == 0), stop=(kc == SC - 1),
                        )
                    nc.scalar.copy(csS[0:1, 4 * i:4 * (i + 1), :], ps)
                return g
            return [ms, mk(0), mk(1)]

        def head(h, qT, kT, vp, msk, csS, attnT, tag):
            """One attention head: scores -> exp -> mask -> PV(+corr) -> norm."""
            po, ch = (h % 2) * DK, h // 2
            sp = spp.tile([P, SC, R], BF16, tag="sp", name=f"sp_{tag}{h}")
            for jj in range(4):
                sc = pss.tile([P, 2, R], F32, tag="sc", name="scps")
                for i in range(2):
                    kc = 2 * jj + i
                    nc.tensor.matmul(
                        sc[:, i, :],
                        lhsT=kT[po:po + DK, ch, kc * P:(kc + 1) * P],
                        rhs=qT[po:po + DK, ch, :], start=True, stop=True,
                    )
                nc.scalar.activation(sp[:, 2 * jj:2 * jj + 2, :], sc[:], AF.Exp)
                nc.vector.scalar_tensor_tensor(
                    out=sp[:, 2 * jj:2 * jj + 2, :],
                    in0=sp[:, 2 * jj:2 * jj + 2, :], scalar=1.0,
                    in1=msk[:, 2 * jj:2 * jj + 2, :],
                    op0=OP.subtract, op1=OP.mult,
                )
            if DBG and h == 0:
                dump(f"dbg_sp0_{tag}", sp[:], BF16)
            pv = psv.tile([P, R], F32, tag="pv", name="pvps")
            for kc in range(SC):
                nc.tensor.matmul(
                    pv[0:DK + 1, :], lhsT=vp[:, kc, h, :], rhs=sp[:, kc, :],
                    start=(kc == 0), stop=False,
                )
            nc.tensor.matmul(
                pv[0:DK + 1, :], lhsT=csS[:, h, :], rhs=ones_row[:, 0:R],
                start=False, stop=True,
            )
            rz = sml.tile([1, R], F32, tag="rz", bufs=1, name=f"rz_{tag}{h}")
            nc.vector.tensor_copy(out=rz, in_=pv[DK:DK + 1, :])
            nc.vector.reciprocal_approx_fast(out=rz, in_=rz)
            if DBG and h == 0:
                dump(f"dbg_rz0_{tag}", rz[:], F32)
            nc.vector.tensor_copy(out=rzb[0:1, :], in_=rz)
            rr = psm.tile([P, R], F32, tag="mm", name="rrps")
            nc.tensor.matmul(
                rr[0:DK, :], lhsT=ones_row[:, 0:DK], rhs=rzb,
                start=True, stop=True,
            )
            rrS = sml.tile([DK, R], BF16, tag="rrS", bufs=1, name=f"rrS_{tag}{h}")
            nc.scalar.copy(rrS, rr[0:DK, :])
            if DBG and h == 0:
                dump(f"dbg_rrS0_{tag}", rrS[:], BF16)
            nc.vector.tensor_mul(attnT[po:po + DK, ch, :], pv[0:DK, :], rrS)

        def outproj(attnT, wo, row_i, resid, xpre_name):
            """xpre [P, RC, D] bf16 = attn @ wo + row + resid."""
            xpre = xnp.tile([P, RC, D], BF16, tag="xpre", name=xpre_name)
            for qm in range(RC):
                ps = psm.tile([P, D], F32, tag="mm", name="ops")
                for kc in range(DC):
                    nc.tensor.matmul(
                        ps, lhsT=attnT[:, kc, qm * P:(qm + 1) * P],
                        rhs=wo[:, kc, :],
                        start=(kc == 0), stop=False,
                    )
                nc.tensor.matmul(
                    ps, lhsT=ones_row[:, 0:P],
                    rhs=rows_sb[:, row_i, :], start=False, stop=True,
                )
                nc.vector.tensor_add(out=xpre[:, qm, :], in0=ps,
                                     in1=resid[:, qm, :])
            return xpre

        def layernorm(xpre, name):
            """In-place: xpre <- (x - mu) * rsqrt(var + eps)   (no affine)."""
            y = xpre
            for qm in range(RC):
                stats = stp.tile([P, 6], F32, tag="stats")
                nc.vector.bn_stats(stats, xpre[:, qm, :])
                mv = stp.tile([P, 2], F32, tag="mv")
                nc.vector.bn_aggr(mv, stats)
                t = stp.tile([P, 1], F32, tag="ln_t")
                nc.scalar.activation(t, mv[:, 1:2], AF.Ln, bias=eps_t)
                rstd = stp.tile([P, 1], F32, tag="ln_rstd")
                nc.scalar.activation(rstd, t, AF.Exp, scale=-0.5)
                nc.vector.tensor_scalar(
                    out=y[:, qm, :], in0=xpre[:, qm, :],
                    scalar1=mv[:, 0:1], scalar2=rstd,
                    op0=OP.subtract, op1=OP.mult,
                )
            return y

        def affine(y, rep_i, name):
            """x_aff = y * g_rep  (bf16, one 2x stt per row-block)."""
            aff = xnp.tile([P, RC, D], BF16, tag="aff", name=name)
            for qm in range(RC):
                nc.vector.tensor_mul(aff[:, qm, :], y[:, qm, :],
                                     reps[:, rep_i, :])
            return aff

        def transpose_nat(x_nat, drain, name):
            """[P, RC, D] natural -> [P, DC, R] transposed via identity mms.
            drain(c, psum_ap) writes channel block c."""
            xT_ = xnp.tile([P, DC, R], BF16, tag="xT", name=name)
            for c in range(DC):
                pt = psm.tile([P, R], F32, tag="mm", name="tps")
                for qm in range(RC):
                    nc.tensor.matmul(
                        pt[:, qm * P:(qm + 1) * P],
                        lhsT=x_nat[:, qm, c * P:(c + 1) * P], rhs=ident,
                        start=(qm == 0), stop=(qm == RC - 1),
                    )
                drain(c, pt, xT_)
            return xT_

        # ================= SA =================
        qT1 = qproj(xq1, sa_w["q"], bq_sa, "qT1")
        kT1 = ktp.tile([P, DC, S], BF16, tag="kT", name="kT1")
        for g in kproj_groups(xq1, sa_w["k"], bk_sa, kT1):
            g()
        vp1 = vpp.tile([P, SC, NH, DK + 1], BF16, tag="vp", name="vp1")
        for g in vproj_groups(xv1, sa_w["v"], rows_sb[:, ROW_BV_SA, :], vp1):
            g()
        cs1 = sml.tile([P, NH, DK + 1], BF16, tag="cs", name="cs1")
        for g in colsum_groups(vp1, cs1):
            g()

        # CA prefetch feeder: emitted interleaved with SA heads
        kT2 = ktp.tile([P, DC, S], BF16, tag="kT", name="kT2")
        vp2 = vpp.tile([P, SC, NH, DK + 1], BF16, tag="vp", name="vp2")
        cs2 = sml.tile([P, NH, DK + 1], BF16, tag="cs", name="cs2")
        feeder = (
            kproj_groups(xq2, ca_w["k"], bk_ca, kT2)
            + vproj_groups(xv2, ca_w["v"], rows_sb[:, ROW_BV_CA, :], vp2)
            + colsum_groups(vp2, cs2)
        )

        attnT1 = atp.tile([P, DC, R], BF16, tag="attnT", name="attnT1")
        for h in range(NH):
            head(h, qT1, kT1, vp1, msa, cs1, attnT1, "sa")
            for _ in range(3):
                if feeder:
                    feeder.pop(0)()
        while feeder:
            feeder.pop(0)()

        if DBG:
            dump("dbg_qT1", qT1[:], BF16)
            dump("dbg_kT1", kT1[:], BF16)
            dump("dbg_vp1", vp1[:], BF16)
            dump("dbg_cs1", cs1[:], BF16)
            dump("dbg_attnT1", attnT1[:], BF16)

        xpre1 = outproj(attnT1, sa_w["o"], ROW_SA, res0, "xpre1")
        y1 = layernorm(xpre1, "y1")
        x1aff = affine(y1, REP_G1, "x1aff")

        def drain_x1q(c, pt, xT_):
            nc.vector.tensor_add(out=xT_[:, c, :], in0=pt,
                                 in1=tpos_t1[:, c, :])
        x1qT = transpose_nat(x1aff, drain_x1q, "x1qT")

        if DBG:
            dump("dbg_xpre1", xpre1[:], BF16)
            dump("dbg_y1", y1[:], BF16)
            dump("dbg_x1aff", x1aff[:], BF16)
            dump("dbg_x1qT", x1qT[:], BF16)

        # ================= CA =================
        qT2 = qproj(x1qT, ca_w["q"], bq_ca, "qT2")
        attnT2 = atp.tile([P, DC, R], BF16, tag="attnT", name="attnT2")
        for h in range(NH):
            head(h, qT2, kT2, vp2, mca, cs2, attnT2, "ca")

        xpre2 = outproj(attnT2, ca_w["o"], ROW_CA, x1aff, "xpre2")
        y2 = layernorm(xpre2, "y2")
        x2aff = affine(y2, REP_G2, "x2aff")

        def drain_x2T(c, pt, xT_):
            nc.scalar.activation(xT_[:, c, :], pt, AF.Identity,
                                 bias=t2pp[:, c:c + 1])
        x2T = transpose_nat(x2aff, drain_x2T, "x2T")

        if DBG:
            dump("dbg_xpre2", xpre2[:], BF16)
            dump("dbg_x2aff", x2aff[:], BF16)
            dump("dbg_x2T", x2T[:], BF16)

        # ================= FFN =================
        psf = []
        for i in range(2):
            pf = pss.tile([P, 2, R], F32, tag="sc", name=f"psf{i}")
            psf.append(pf)

        def psf_ap(qm):
            return psf[qm // 2][:, qm % 2, :]

        for hc in range(HC):
            ph = psm.tile([P, R], F32, tag="mm", name="phps")
            for kc in range(DC):
                nc.tensor.matmul(
                    ph, lhsT=w1b[:, kc, hc * P:(hc + 1) * P], rhs=x2T[:, kc, :],
                    start=(kc == 0), stop=(kc == DC - 1),
                )
            hcb = hcp.tile([P, R], BF16, tag="hc")
            nc.scalar.activation(hcb, ph, AF.Relu, bias=b1pp[:, hc:hc + 1])
            for qm in range(RC):
                nc.tensor.matmul(
                    psf_ap(qm), lhsT=hcb[:, qm * P:(qm + 1) * P],
                    rhs=w2b[:, hc, :],
                    start=(hc == 0), stop=False,
                )
        xpre3 = xnp.tile([P, RC, D], BF16, tag="xpre", name="xpre3")
        for qm in range(RC):
            nc.tensor.matmul(
                psf_ap(qm), lhsT=ones_row[:, 0:P],
                rhs=rows_sb[:, ROW_FFN, :], start=False, stop=True,
            )
            nc.vector.tensor_add(out=xpre3[:, qm, :], in0=psf_ap(qm),
                                 in1=x2aff[:, qm, :])
        if DBG:
            dump("dbg_xpre3", xpre3[:], BF16)

        y3 = layernorm(xpre3, "y3")
        for qm in range(RC):
            ot = atp.tile([P, D], F32, tag="attnT", name="ot")
            nc.vector.tensor_mul(ot, y3[:, qm, :], reps[:, REP_G3, :])
            nc.vector.tensor_add(out=ot, in0=ot, in1=reps[:, REP_T3, :])
            nc.sync.dma_start(out_d[:, qm, :], ot)


# ----------------------------------------------------------------------------
_NC_CACHE = None


def _get_nc():
    global _NC_CACHE
    if _NC_CACHE is None:
        _NC_CACHE = build_nc()
    return _NC_CACHE


def _layP(a, nchunk):
    """[nchunk*P, ...] -> [P, nchunk, ...] contiguous."""
    a = a.reshape(nchunk, P, *a.shape[1:])
    return np.ascontiguousarray(a.transpose(1, 0, *range(2, a.ndim)))


def make_in_maps(inputs):
    """Shard + preprocess full inputs -> 8 per-core in_maps (all bf16-ready)."""
    import ml_dtypes

    bf = ml_dtypes.bfloat16
    t = {k: np.asarray(v, dtype=np.float32) if np.asarray(v).dtype != np.int32
         else np.asarray(v) for k, v in inputs.items()}
    in_maps = []
    for core in range(8):
        b, half = core // 2, core % 2
        rows = slice(half * R, half * R + R)
        xq_sa = np.roll((t["tgt"][b] + t["tgt_pos"][b]).T, -half * R, axis=1)
        xv_sa = np.roll(t["tgt"][b].T, -half * R, axis=1)
        xq_ca = (t["memory"][b] + t["memory_pos"][b]).T
        xv_ca = t["memory"][b].T
        m_sa = np.roll(t["tgt_mask"][b, rows, :], -half * R, axis=1).T
        m_ca = t["memory_mask"][b, rows, :].T
        m = {
            "xq_sa": _layP(xq_sa.astype(bf), DC),
            "xv_sa": _layP(xv_sa.astype(bf), DC),
            "xq_ca": _layP(xq_ca.astype(bf), DC),
            "xv_ca": _layP(xv_ca.astype(bf), DC),
            "res0": _layP(t["tgt"][b, rows, :].astype(bf), RC),
            "tpos_t1": _layP(
                (t["tgt_pos"][b, rows, :].T + t["ln1_b"][:, None]).astype(bf), DC),
            "m_sa": _layP(m_sa.astype(np.float32).astype(bf), SC),
            "m_ca": _layP(m_ca.astype(np.float32).astype(bf), SC),
            "w1": _layP(t["mlp_w1"].astype(bf), DC),
            "w2": _layP(t["mlp_w2"].astype(bf), HC),
            "t2pp": np.ascontiguousarray(t["ln2_b"].reshape(DC, P).T),
            "b1pp": np.ascontiguousarray(t["mlp_b1"].reshape(HC, P).T),
            "rows_d": np.concatenate([
                np.concatenate([
                    t["sa_bo"],
                    t["ca_bo"] + t["ln1_b"],
                    t["mlp_b2"] + t["ln2_b"],
                    t["sa_bv"],
                    t["ca_bv"],
                ])[None, :],
                np.zeros((127, 5 * D), np.float32),
            ]).astype(bf),
            "grep_d": np.concatenate([
                t["ln1_g"], t["ln2_g"], t["ln3_g"], t["ln3_b"],
            ])[None, :].astype(bf),
        }
        for pre in ("sa", "ca"):
            for w in "qkvo":
                scl = ISQ if w == "q" else 1.0
                m[f"w_{pre}_{w}"] = _layP((t[f"{pre}_w{w}"] * scl).astype(bf), DC)
            m[f"bq_{pre}"] = np.ascontiguousarray(
                (t[f"{pre}_bq"] * ISQ).reshape(DC, P).T.astype(np.float32))
            m[f"bk_{pre}"] = np.ascontiguousarray(
                t[f"{pre}_bk"].reshape(DC, P).T.astype(np.float32))
        in_maps.append(m)
    return in_maps


def gather_out(results):
    out = np.zeros((4, S, D), np.float32)
    for core in range(8):
        b, half = core // 2, core % 2
        o = np.asarray(results[core]["out"], dtype=np.float32)  # [P, RC, D]
        out[b, half * R:half * R + R, :] = o.transpose(1, 0, 2).reshape(R, D)
    return out


def kernel(**inputs):
    from concourse import bass_utils

    nc = _get_nc()
    in_maps = make_in_maps(inputs)
    res = bass_utils.run_bass_kernel_spmd(nc, in_maps, core_ids=list(range(8)))
    return gather_out(res.results)
